# revision 8
# baseline (speedup 1.0000x reference)
"""Trainium2 Bass kernel for nn_AttentionLayer (linear attention, conv1x1 projections).

Math (per batch b, with x flattened to [C=512, L=4096]):
    QP = Wq @ x + bq ; Q = elu(QP)+1
    KP = Wk @ x + bk ; K = elu(KP)+1
    VP = Wv @ x + bv          (reference divides by L here and multiplies by L
                               at the end -- exact cancellation, so we drop both)
    per head h (64 channels each):
        KV_h   = K_h @ V_h^T                  [64, 64]
        Ksum_h = K_h @ ones                   [64]
        S_h[l] = Ksum_h . Q_h[:, l]
        out_h  = (KV_h^T @ Q_h) / S_h         (eps=1e-6 negligible vs S~1e5)
    y = Wo @ out + bo

Distribution: data-parallel over batch, 1 batch per NeuronCore (8 cores).
All matmuls run in bf16 (f32 PSUM accumulate).  elu(x)+1 is computed exactly as
min(exp(x), 1) + relu(x)  (uses exp(x) >= x+1, equality at 0).

Layouts on chip:
    Q   [c, l]  (normal)      -- rhs for S/out matmuls, lhsT = per-head blocks
    K^T [l, c]  (transposed)  -- produced directly by using x-chunks as lhsT
    V^T [l, c]  (transposed)
    KV_bd  [c-chunk, 128]: per 128-chunk m, block-diag(KV_2m, KV_2m+1)
    KsumRep[c-chunk, 128]: block-diag(Ksum_2m 1^T, Ksum_2m+1 1^T) -> S replicated
        to all 128 partitions so the reciprocal multiply needs no partition
        broadcast.
"""

import sys

import numpy as np

if "/opt/trn_rl_repo" not in sys.path:
    sys.path.insert(0, "/opt/trn_rl_repo")

import ml_dtypes

BF16 = ml_dtypes.bfloat16

C = 512
L = 4096
NB = 8          # batches == cores
NCC = 4         # 128-row chunks of C
NL512 = 8       # 512-col chunks of L
NL128 = 32      # 128-col chunks of L

_CACHE = {}


def _build_nc(debug_outputs=False):
    import concourse.bass as bass  # noqa: F401
    import concourse.tile as tile
    from concourse import bacc, mybir

    f32 = mybir.dt.float32
    bf16 = mybir.dt.bfloat16
    AF = mybir.ActivationFunctionType
    OP = mybir.AluOpType

    nc = bacc.Bacc("TRN2", target_bir_lowering=False, debug=False,
                   enable_asserts=False, num_devices=NB)

    dbg = {}
    if debug_outputs:
        dbg["Q"] = nc.dram_tensor("dbg_Q", [128, NCC, L], bf16, kind="ExternalOutput")
        dbg["Kt"] = nc.dram_tensor("dbg_Kt", [128, NL128, C], bf16, kind="ExternalOutput")
        dbg["Vt"] = nc.dram_tensor("dbg_Vt", [128, NL128, NCC, 129], bf16, kind="ExternalOutput")
        dbg["KVbd"] = nc.dram_tensor("dbg_KVbd", [128, NCC, 128], bf16, kind="ExternalOutput")
        dbg["KsumRep"] = nc.dram_tensor("dbg_KsumRep", [128, NCC, 128], bf16, kind="ExternalOutput")
        dbg["Om"] = nc.dram_tensor("dbg_Om", [128, NCC, L], bf16, kind="ExternalOutput")

    x_d = nc.dram_tensor("x", [C, L], bf16, kind="ExternalInput")
    wq_d = nc.dram_tensor("wqT", [C, C], bf16, kind="ExternalInput")
    wk_d = nc.dram_tensor("wkT", [C, C], bf16, kind="ExternalInput")
    wv_d = nc.dram_tensor("wvT", [C, C], bf16, kind="ExternalInput")
    wo_d = nc.dram_tensor("woT", [C, C], bf16, kind="ExternalInput")
    bq_d = nc.dram_tensor("bqT", [128, NCC], f32, kind="ExternalInput")
    bo_d = nc.dram_tensor("boT", [128, NCC], f32, kind="ExternalInput")
    bkb_d = nc.dram_tensor("bkb", [128, C], f32, kind="ExternalInput")
    bvb_d = nc.dram_tensor("bvb", [128, C], f32, kind="ExternalInput")
    out_d = nc.dram_tensor("out", [C, L], f32, kind="ExternalOutput")

    x_ap = x_d.ap().rearrange("(cc p) l -> p cc l", p=128)   # [128, 4, L]
    out_ap = out_d.ap()

    from contextlib import ExitStack

    with tile.TileContext(nc) as tc:
        with ExitStack() as stack:
            const = stack.enter_context(tc.tile_pool(name="const", bufs=1))
            big = stack.enter_context(tc.tile_pool(name="big", bufs=1))
            xin = stack.enter_context(tc.tile_pool(name="xin", bufs=2))
            ev = stack.enter_context(tc.tile_pool(name="ev", bufs=3))
            # ---- constants (load order = first-use order) ----
            wq_sb = const.tile([128, NCC, C], bf16)
            wk_sb = const.tile([128, NCC, C], bf16)
            wv_sb = const.tile([128, NCC, C], bf16)
            wo_sb = const.tile([128, NCC, C], bf16)
            bq_sb = const.tile([128, NCC], f32)
            bo_sb = const.tile([128, NCC], f32)
            bkb_sb = const.tile([128, C], f32)
            bvb_sb = const.tile([128, C], f32)
            # Startup loads split across both HWDGE queues (SP + ACT): the
            # K-projection path (x j=0 interleaved with wk, then bkb) on SP,
            # everything else on the ACT queue.  The first kp matmul only
            # needs (x cc=0, wk cc=0), so interleaving starts PE ~1.3us in.
            xt0 = xin.tile([128, NCC, 512], bf16, name="xt0", tag="xt")
            for cc in range(NCC):
                nc.sync.dma_start(out=xt0[:, cc, :], in_=x_ap[:, cc, 0:512])
                nc.sync.dma_start(out=wk_sb[:, cc, :],
                                  in_=wk_d.ap()[128 * cc:128 * (cc + 1), :])
            nc.sync.dma_start(out=bkb_sb, in_=bkb_d.ap())
            for cc in range(NCC):
                nc.scalar.dma_start(out=wv_sb[:, cc, :],
                                    in_=wv_d.ap()[128 * cc:128 * (cc + 1), :])
            nc.scalar.dma_start(out=bvb_sb, in_=bvb_d.ap())
            for cc in range(NCC):
                nc.scalar.dma_start(out=wq_sb[:, cc, :],
                                    in_=wq_d.ap()[128 * cc:128 * (cc + 1), :])
            nc.scalar.dma_start(out=bq_sb, in_=bq_d.ap())
            for cc in range(NCC):
                nc.scalar.dma_start(out=wo_sb[:, cc, :],
                                    in_=wo_d.ap()[128 * cc:128 * (cc + 1), :])
            nc.scalar.dma_start(out=bo_sb, in_=bo_d.ap())
            ones128_sb = const.tile([128, 64], bf16)
            nc.vector.memset(ones128_sb, 1.0)

            # ---- persistent activations ----
            Q_sb = big.tile([128, NCC, L], bf16)     # [c, l] normal
            Kt_sb = big.tile([128, NL128, C], bf16)  # [l, c] transposed
            # V^T stored per m-chunk with a trailing ones column: [l, m, 129]
            # so the KV matmul's 129th output column accumulates Ksum in the
            # SAME psum chain (start=True zeroes a whole 2KB bank, so each
            # chain needs a private bank and no sibling chains).
            Vt_sb = big.tile([128, NL128, NCC, 129], bf16)
            nc.vector.memset(Vt_sb[:, :, :, 128:129], 1.0)
            Om_sb = big.tile([128, NCC, L], bf16)    # attention out, [c, l]
            KVbd_sb = const.tile([128, NCC, 128], bf16)
            KsumRep_sb = const.tile([128, NCC, 128], bf16)
            ksum_sb = const.tile([128, NCC], f32)
            # zero the off-diagonal blocks up front (not data dependent)
            for m in range(NCC):
                nc.vector.memset(KVbd_sb[0:64, m, 64:128], 0.0)
                nc.vector.memset(KVbd_sb[64:128, m, 0:64], 0.0)
                nc.vector.memset(KsumRep_sb[0:64, m, 64:128], 0.0)
                nc.vector.memset(KsumRep_sb[64:128, m, 0:64], 0.0)

            def bcast_pair(ap):
                """View an AP ([128, ...]) with a broadcast pair dim inserted
                after partitions: [128, 2(step 0), ...]."""
                return bass.AP(tensor=ap.tensor, offset=ap.offset,
                               ap=[list(ap.ap[0]), [0, 2],
                                   *[list(d) for d in ap.ap[1:]]])

            # ================= phase 1: projections + KV accumulation =========
            with ExitStack() as p1stack:
                # PSUM (8 banks): kp x2 + vp x2 (4) + qp pair (2)
                #               + KV accumulators paired 2-per-bank (2)
                pkv = p1stack.enter_context(tc.tile_pool(name="pkv", bufs=2, space="PSUM"))
                pq = p1stack.enter_context(tc.tile_pool(name="pq", bufs=1, space="PSUM"))
                pacc = p1stack.enter_context(tc.tile_pool(name="pacc", bufs=1, space="PSUM"))
                # Two chains share one bank: only the temporally-first matmul in
                # a bank carries start=True (start zeroes the whole 2KB bank),
                # only the last carries stop=True.  CoreSim's psum-group checker
                # verifies the ordering holds.
                KV_ps = [pacc.tile([128, 129], f32, tag=f"kvacc{g}", name=f"kv_ps{g}")
                         for g in range(4)]

                bvb4 = bvb_sb[:, :].rearrange("p (m c) -> p m c", m=NCC)
                # Q's final combine is deferred one half-iteration: it is only
                # consumed in phase 2, and emitting it late keeps the DVE FIFO
                # clear so the next half's psum-releasing adds run sooner.
                pending_qstt = []
                for j in range(NL512):
                    if j == 0:
                        xt = xt0
                    else:
                        xt = xin.tile([128, NCC, 512], bf16, name="xt", tag="xt")
                        nc.sync.dma_start(out=xt,
                                          in_=x_ap[:, :, 512 * j:512 * (j + 1)])

                    def kv_half(j, half, xt):
                        li0 = 4 * j + 2 * half
                        for jj2 in range(2):
                            li = li0 + jj2
                            lf = 128 * (2 * half + jj2)
                            # kp and vp in independently-released single banks
                            kp = pkv.tile([128, 512], f32, tag="kp", name="kp", bufs=1)
                            vp = pkv.tile([128, 512], f32, tag="vp", name="vp", bufs=1)
                            for cc in range(NCC):
                                xs = xt[:, cc, lf:lf + 128]
                                nc.tensor.matmul(kp, xs, wk_sb[:, cc, :],
                                                 start=(cc == 0), stop=(cc == NCC - 1))
                            for cc in range(NCC):
                                xs = xt[:, cc, lf:lf + 128]
                                nc.tensor.matmul(vp, xs, wv_sb[:, cc, :],
                                                 start=(cc == 0), stop=(cc == NCC - 1))
                            # K^T = elu(kp + bk) + 1 = min(exp(t),1) + relu(t)
                            # relu + combine run on GpSimd (SBUF-only engine,
                            # otherwise idle) to keep DVE/ACT off the critical
                            # path; exp stays on ACT (table op).
                            t0 = ev.tile([128, 512], f32, tag="t0", bufs=3, name="t0")
                            e0 = ev.tile([128, 512], bf16, tag="e0", bufs=4, name="e0")
                            r0 = ev.tile([128, 512], bf16, tag="r0", bufs=4, name="r0")
                            nc.vector.tensor_add(t0, kp, bkb_sb)
                            # V^T = vp + bv  (written into the 129-strided layout)
                            nc.vector.tensor_add(
                                Vt_sb[:, li, :, 0:128],
                                vp.rearrange("p (m c) -> p m c", m=NCC), bvb4)
                            nc.scalar.activation(e0, t0, AF.Exp)
                            nc.gpsimd.tensor_scalar_max(r0, t0, 0.0)
                            nc.vector.scalar_tensor_tensor(
                                Kt_sb[:, li, :], e0, 1.0, r0, OP.min, OP.add)
                            # KV (+ Ksum in col 128) accumulation over l
                            for m in range(NCC):
                                ks = Kt_sb[:, li, 128 * m:128 * (m + 1)]
                                nc.tensor.matmul(
                                    KV_ps[m], ks, Vt_sb[:, li, m, :],
                                    start=(li == 0), stop=(li == NL128 - 1))

                    def q_proj(j, op2, xt):
                        # Q projection, one oi-pair (spreads ACT load).  Two
                        # single-bank psum tags rotate so oi+1's matmuls can
                        # run while oi's ACT evictions still read their bank.
                        eq = ev.tile([128, 2, 512], bf16, tag="e0", bufs=4, name="eq")
                        rq = ev.tile([128, 2, 512], bf16, tag="r0", bufs=4, name="rq")
                        for oi2 in range(2):
                            oi = 2 * op2 + oi2
                            qp = pq.tile([128, 512], f32, tag=f"qp{oi2}",
                                         name=f"qp{oi2}")
                            for cc in range(NCC):
                                nc.tensor.matmul(
                                    qp,
                                    wq_sb[:, cc, 128 * oi:128 * (oi + 1)],
                                    xt[:, cc, :],
                                    start=(cc == 0), stop=(cc == NCC - 1))
                            nc.scalar.activation(eq[:, oi2, :], qp,
                                                 AF.Exp, bias=bq_sb[:, oi:oi + 1])
                            nc.scalar.activation(rq[:, oi2, :], qp,
                                                 AF.Relu, bias=bq_sb[:, oi:oi + 1])
                            if pending_qstt:
                                nc.vector.scalar_tensor_tensor(*pending_qstt.pop())
                            pending_qstt.append(
                                (Q_sb[:, oi, 512 * j:512 * (j + 1)],
                                 eq[:, oi2, :], 1.0, rq[:, oi2, :],
                                 OP.min, OP.add))

                    if j < NL512 - 1:
                        for half in range(2):
                            kv_half(j, half, xt)
                            q_proj(j, half, xt)
                    else:
                        # last chunk: Q first, so the dangling K/V eviction
                        # chain into the final KV matmuls is as short as
                        # possible before the phase-2 transition
                        q_proj(j, 0, xt)
                        q_proj(j, 1, xt)
                        kv_half(j, 0, xt)
                        kv_half(j, 1, xt)

                # ---- evict KV (block-diag) and Ksum ----
                for m in range(NCC):
                    kv_m = KV_ps[m]
                    nc.vector.tensor_copy(KVbd_sb[0:64, m, 0:64], kv_m[0:64, 0:64])
                    nc.vector.tensor_copy(KVbd_sb[64:128, m, 64:128],
                                          kv_m[64:128, 64:128])
                    nc.vector.tensor_copy(ksum_sb[:, m:m + 1], kv_m[:, 128:129])
                for m in range(NCC):
                    nc.vector.tensor_scalar_mul(
                        KsumRep_sb[0:64, m, 0:64], ones128_sb[0:64, :],
                        ksum_sb[0:64, m:m + 1])
                    nc.vector.tensor_scalar_mul(
                        KsumRep_sb[64:128, m, 64:128], ones128_sb[64:128, :],
                        ksum_sb[64:128, m:m + 1])
                # the last Q combine is only consumed by phase-2 j=7 -- flush
                # it AFTER the KV eviction chain so it doesn't delay S_b(j=0)
                if pending_qstt:
                    nc.vector.scalar_tensor_tensor(*pending_qstt.pop())

            if debug_outputs:
                nc.sync.dma_start(out=dbg["Q"].ap(), in_=Q_sb)
                nc.sync.dma_start(out=dbg["Kt"].ap(), in_=Kt_sb)
                nc.sync.dma_start(out=dbg["Vt"].ap(), in_=Vt_sb)
                nc.sync.dma_start(out=dbg["KVbd"].ap(), in_=KVbd_sb)
                nc.sync.dma_start(out=dbg["KsumRep"].ap(), in_=KsumRep_sb)

            # ================= phase 2: attention out + O-projection ==========
            # The out matmuls read Q directly (no dependency on the reciprocal
            # chain); Z is applied by the DVE eviction multiply, so the PE
            # stream never waits on DVE except via psum slot reuse.
            with ExitStack() as p2stack:
                # PSUM: sb pair (2) + out pair (2) + y pair x2 bufs (4) = 8
                p2 = p2stack.enter_context(tc.tile_pool(name="p2", bufs=1, space="PSUM"))
                p2o = p2stack.enter_context(tc.tile_pool(name="p2o", bufs=2, space="PSUM"))
                zbp = p2stack.enter_context(tc.tile_pool(name="zbp", bufs=2))
                ytp = p2stack.enter_context(tc.tile_pool(name="ytp", bufs=2))

                def y_block(j, fine=False):
                    lsl = slice(512 * j, 512 * (j + 1))
                    for op2 in range(2):
                        yt = ytp.tile([128, 2, 512], f32, name="yt")
                        for oi2 in range(2):
                            oi = 2 * op2 + oi2
                            yp = p2o.tile([128, 512], f32, tag=f"y{oi2}",
                                          name="yp", bufs=2)
                            for vi in range(NCC):
                                nc.tensor.matmul(
                                    yp,
                                    wo_sb[:, vi, 128 * oi:128 * (oi + 1)],
                                    Om_sb[:, vi, lsl],
                                    start=(vi == 0), stop=(vi == NCC - 1))
                            nc.scalar.activation(yt[:, oi2, :], yp,
                                                 AF.Identity, bias=bo_sb[:, oi:oi + 1])
                            if fine:  # drain the tail with per-oi DMAs
                                nc.sync.dma_start(
                                    out=out_ap[128 * oi:128 * (oi + 1), lsl],
                                    in_=yt[:, oi2, :])
                        if not fine:
                            nc.sync.dma_start(
                                out=out_ap[256 * op2:256 * (op2 + 1), lsl].rearrange(
                                    "(two p) l -> p two l", p=128),
                                in_=yt)

                # y-block runs one j behind so its matmuls fill the PE while the
                # next j's recip/qz DVE chain is in flight.
                for j in range(NL512):
                    lsl = slice(512 * j, 512 * (j + 1))
                    for mp in range(2):
                        sbp = p2.tile([128, 2, 512], f32, tag="sb", name="sbp")
                        outp = p2.tile([128, 2, 512], f32, tag="out", name="outp")
                        for m2 in range(2):
                            m = 2 * mp + m2
                            nc.tensor.matmul(sbp[:, m2, :], KsumRep_sb[:, m, :],
                                             Q_sb[:, m, lsl], start=True, stop=True)
                            nc.tensor.matmul(outp[:, m2, :], KVbd_sb[:, m, :],
                                             Q_sb[:, m, lsl], start=True, stop=True)
                        zb = zbp.tile([128, 2, 512], f32)
                        # approx reciprocal (~18 bits, S~1e5 so no edge cases)
                        # is ~5x cheaper on DVE than the exact reciprocal()
                        nc.vector.reciprocal_approx_fast(zb, sbp)
                        nc.vector.tensor_mul(Om_sb[:, 2 * mp:2 * mp + 2, lsl],
                                             outp, zb)
                    if j > 0:
                        y_block(j - 1)
                y_block(NL512 - 1, fine=True)
                if debug_outputs:
                    nc.sync.dma_start(out=dbg["Om"].ap(), in_=Om_sb)

    nc.compile()
    return nc


def _get_nc():
    if "nc" not in _CACHE:
        _CACHE["nc"] = _build_nc()
    return _CACHE["nc"]


def _make_in_maps(inputs):
    x = np.asarray(inputs["x"], dtype=np.float32)
    wq = np.asarray(inputs["wq"], dtype=np.float32)
    wk = np.asarray(inputs["wk"], dtype=np.float32)
    wv = np.asarray(inputs["wv"], dtype=np.float32)
    wo = np.asarray(inputs["wo"], dtype=np.float32)
    bq = np.asarray(inputs["bq"], dtype=np.float32)
    bk = np.asarray(inputs["bk"], dtype=np.float32)
    bv = np.asarray(inputs["bv"], dtype=np.float32)
    bo = np.asarray(inputs["bo"], dtype=np.float32)

    shared = {
        "wqT": np.ascontiguousarray(wq.T).astype(BF16),
        "wkT": np.ascontiguousarray(wk.T).astype(BF16),
        "wvT": np.ascontiguousarray(wv.T).astype(BF16),
        "woT": np.ascontiguousarray(wo.T).astype(BF16),
        "bqT": np.ascontiguousarray(bq.reshape(NCC, 128).T),
        "boT": np.ascontiguousarray(bo.reshape(NCC, 128).T),
        "bkb": np.ascontiguousarray(np.broadcast_to(bk, (128, C))),
        "bvb": np.ascontiguousarray(np.broadcast_to(bv, (128, C))),
    }
    in_maps = []
    for b in range(NB):
        m = dict(shared)
        m["x"] = np.ascontiguousarray(x[b].reshape(C, L)).astype(BF16)
        in_maps.append(m)
    return in_maps


def _run(inputs, trace=False):
    from concourse.bass_utils import run_bass_kernel_spmd

    nc = _get_nc()
    in_maps = _make_in_maps(inputs)
    res = run_bass_kernel_spmd(nc, in_maps, core_ids=list(range(NB)), trace=trace)
    outs = np.stack([np.asarray(res.results[b]["out"], dtype=np.float32)
                     for b in range(NB)])
    y = outs.reshape(NB, C, 64, 64)
    return y, res


def kernel(**inputs) -> np.ndarray:
    y, _ = _run(inputs, trace=False)
    return y



# revision 15
# speedup vs baseline: 2.2084x; 2.2084x over previous
"""Trainium2 Bass kernel for nn_AttentionLayer (linear attention, conv1x1 projections).

Math (per batch b, with x flattened to [C=512, L=4096]):
    QP = Wq @ x + bq ; Q = elu(QP)+1
    KP = Wk @ x + bk ; K = elu(KP)+1
    VP = Wv @ x + bv          (reference divides by L here and multiplies by L
                               at the end -- exact cancellation, so we drop both)
    per head h (64 channels each):
        KV_h   = K_h @ V_h^T                  [64, 64]
        Ksum_h = K_h @ ones                   [64]
        S_h[l] = Ksum_h . Q_h[:, l]
        out_h  = (KV_h^T @ Q_h) / S_h         (eps=1e-6 negligible vs S~1e5)
    y = Wo @ out + bo

Distribution: data-parallel over batch, 1 batch per NeuronCore (8 cores).
All matmuls run in bf16 (f32 PSUM accumulate).  elu(x)+1 is computed exactly as
min(exp(x), 1) + relu(x)  (uses exp(x) >= x+1, equality at 0).

Layouts on chip:
    Q   [c, l]  (normal)      -- rhs for S/out matmuls, lhsT = per-head blocks
    K^T [l, c]  (transposed)  -- produced directly by using x-chunks as lhsT
    V^T [l, c]  (transposed)
    KV_bd  [c-chunk, 128]: per 128-chunk m, block-diag(KV_2m, KV_2m+1)
    KsumRep[c-chunk, 128]: block-diag(Ksum_2m 1^T, Ksum_2m+1 1^T) -> S replicated
        to all 128 partitions so the reciprocal multiply needs no partition
        broadcast.
"""

import sys

import numpy as np

if "/opt/trn_rl_repo" not in sys.path:
    sys.path.insert(0, "/opt/trn_rl_repo")

import ml_dtypes

BF16 = ml_dtypes.bfloat16

C = 512
L = 4096
NB = 8          # batches == cores
NCC = 4         # 128-row chunks of C
NL512 = 8       # 512-col chunks of L
NL128 = 32      # 128-col chunks of L

_CACHE = {}


def _build_nc(debug_outputs=False):
    import concourse.bass as bass  # noqa: F401
    import concourse.tile as tile
    from concourse import bacc, mybir

    f32 = mybir.dt.float32
    bf16 = mybir.dt.bfloat16
    AF = mybir.ActivationFunctionType
    OP = mybir.AluOpType

    nc = bacc.Bacc("TRN2", target_bir_lowering=False, debug=False,
                   enable_asserts=False, num_devices=NB)

    dbg = {}
    if debug_outputs:
        dbg["Q"] = nc.dram_tensor("dbg_Q", [128, NCC, L], bf16, kind="ExternalOutput")
        dbg["Kt"] = nc.dram_tensor("dbg_Kt", [128, NL128, C], bf16, kind="ExternalOutput")
        dbg["Vt"] = nc.dram_tensor("dbg_Vt", [128, NL128, NCC, 129], bf16, kind="ExternalOutput")
        dbg["KVbd"] = nc.dram_tensor("dbg_KVbd", [128, NCC, 128], bf16, kind="ExternalOutput")
        dbg["KsumRep"] = nc.dram_tensor("dbg_KsumRep", [128, NCC, 128], bf16, kind="ExternalOutput")
        dbg["Om"] = nc.dram_tensor("dbg_Om", [128, NCC, L], bf16, kind="ExternalOutput")

    x_d = nc.dram_tensor("x", [C, L], bf16, kind="ExternalInput")
    wq_d = nc.dram_tensor("wqT", [C, C], bf16, kind="ExternalInput")
    wk_d = nc.dram_tensor("wkT", [C, C], bf16, kind="ExternalInput")
    wv_d = nc.dram_tensor("wvT", [C, C], bf16, kind="ExternalInput")
    wo_d = nc.dram_tensor("woT", [C, C], bf16, kind="ExternalInput")
    bq_d = nc.dram_tensor("bqT", [128, NCC], f32, kind="ExternalInput")
    bo_d = nc.dram_tensor("boT", [128, NCC], f32, kind="ExternalInput")
    bkb_d = nc.dram_tensor("bkb", [128, C], f32, kind="ExternalInput")
    bvb_d = nc.dram_tensor("bvb", [128, C], f32, kind="ExternalInput")
    out_d = nc.dram_tensor("out", [C, L], f32, kind="ExternalOutput")

    x_ap = x_d.ap().rearrange("(cc p) l -> p cc l", p=128)   # [128, 4, L]
    out_ap = out_d.ap()

    from contextlib import ExitStack

    with tile.TileContext(nc) as tc:
        with ExitStack() as stack:
            const = stack.enter_context(tc.tile_pool(name="const", bufs=1))
            big = stack.enter_context(tc.tile_pool(name="big", bufs=1))
            xin = stack.enter_context(tc.tile_pool(name="xin", bufs=2))
            ev = stack.enter_context(tc.tile_pool(name="ev", bufs=3))
            # ---- constants (load order = first-use order) ----
            wq_sb = const.tile([128, NCC, C], bf16)
            wk_sb = const.tile([128, NCC, C], bf16)
            wv_sb = const.tile([128, NCC, C], bf16)
            wo_sb = const.tile([128, NCC, C], bf16)
            bq_sb = const.tile([128, NCC], f32)
            bo_sb = const.tile([128, NCC], f32)
            bkb_sb = const.tile([128, C], f32)
            bvb_sb = const.tile([128, C], f32)
            # Startup loads split across both HWDGE queues (SP + ACT): the
            # K-projection path (x j=0 interleaved with wk, then bkb) on SP,
            # everything else on the ACT queue.  The first kp matmul only
            # needs (x cc=0, wk cc=0), so interleaving starts PE ~1.3us in.
            xt0 = xin.tile([128, NCC, 512], bf16, name="xt0", tag="xt")
            for cc in range(NCC):
                nc.sync.dma_start(out=xt0[:, cc, :], in_=x_ap[:, cc, 0:512])
                nc.sync.dma_start(out=wk_sb[:, cc, :],
                                  in_=wk_d.ap()[128 * cc:128 * (cc + 1), :])
            nc.sync.dma_start(out=bkb_sb, in_=bkb_d.ap())
            # Remaining weight loads ride the GpSimd (Pool) queue: the engine
            # is otherwise idle and its dma dispatch doesn't steal ACT-queue
            # time from the activation stream.
            for cc in range(NCC):
                nc.gpsimd.dma_start(out=wv_sb[:, cc, :],
                                    in_=wv_d.ap()[128 * cc:128 * (cc + 1), :])
            nc.gpsimd.dma_start(out=bvb_sb, in_=bvb_d.ap())
            for cc in range(NCC):
                nc.gpsimd.dma_start(out=wq_sb[:, cc, :],
                                    in_=wq_d.ap()[128 * cc:128 * (cc + 1), :])
            nc.gpsimd.dma_start(out=bq_sb, in_=bq_d.ap())
            for cc in range(NCC):
                nc.gpsimd.dma_start(out=wo_sb[:, cc, :],
                                    in_=wo_d.ap()[128 * cc:128 * (cc + 1), :])
            nc.gpsimd.dma_start(out=bo_sb, in_=bo_d.ap())
            ones128_sb = const.tile([128, 64], bf16)
            nc.vector.memset(ones128_sb, 1.0)

            # ---- persistent activations ----
            Q_sb = big.tile([128, NCC, L], bf16)     # [c, l] normal
            Kt_sb = big.tile([128, NL128, C], bf16)  # [l, c] transposed
            # V^T stored per m-chunk with a trailing ones column: [l, m, 129]
            # so the KV matmul's 129th output column accumulates Ksum in the
            # SAME psum chain (start=True zeroes a whole 2KB bank, so each
            # chain needs a private bank and no sibling chains).
            Vt_sb = big.tile([128, NL128, NCC, 129], bf16)
            nc.vector.memset(Vt_sb[:, :, :, 128:129], 1.0)
            Om_sb = big.tile([128, NCC, L], bf16)    # attention out, [c, l]
            KVbd_sb = const.tile([128, NCC, 128], bf16)
            KsumRep_sb = const.tile([128, NCC, 128], bf16)
            ksum_sb = const.tile([128, NCC], f32)
            # zero the off-diagonal blocks up front (not data dependent)
            for m in range(NCC):
                nc.vector.memset(KVbd_sb[0:64, m, 64:128], 0.0)
                nc.vector.memset(KVbd_sb[64:128, m, 0:64], 0.0)
                nc.vector.memset(KsumRep_sb[0:64, m, 64:128], 0.0)
                nc.vector.memset(KsumRep_sb[64:128, m, 0:64], 0.0)

            def bcast_pair(ap):
                """View an AP ([128, ...]) with a broadcast pair dim inserted
                after partitions: [128, 2(step 0), ...]."""
                return bass.AP(tensor=ap.tensor, offset=ap.offset,
                               ap=[list(ap.ap[0]), [0, 2],
                                   *[list(d) for d in ap.ap[1:]]])

            # ================= phase 1: projections + KV accumulation =========
            with ExitStack() as p1stack:
                # PSUM (8 banks): kp x2 + vp x2 (4) + qp pair (2)
                #               + KV accumulators paired 2-per-bank (2)
                pkv = p1stack.enter_context(tc.tile_pool(name="pkv", bufs=2, space="PSUM"))
                pq = p1stack.enter_context(tc.tile_pool(name="pq", bufs=1, space="PSUM"))
                pacc = p1stack.enter_context(tc.tile_pool(name="pacc", bufs=1, space="PSUM"))
                # Two chains genuinely share one bank (chain g at 1KB offset
                # g*256 f32): only the temporally-first matmul in a bank
                # carries start=True -- start clears the has_written bits of
                # the WHOLE 2KB bank, so the second chain's first matmul must
                # NOT carry start or it would wipe its bank-sibling's li=0
                # contribution.  [128,129] at offsets 0/1024/2048/3072 never
                # crosses a bank boundary, keeping each matmul single-bank.
                KV_all = pacc.tile([128, 4, 256], f32, tag="kvacc", name="kv_all")
                KV_ps = [KV_all[:, g, 0:129] for g in range(4)]

                bvb4 = bvb_sb[:, :].rearrange("p (m c) -> p m c", m=NCC)
                bkb2 = bcast_pair(bkb_sb[:, :])
                bvb4x2 = bcast_pair(bvb4)
                # Q's final combine is deferred one half-iteration: it is only
                # consumed in phase 2, and emitting it late keeps the DVE FIFO
                # clear so the next half's psum-releasing adds run sooner.
                pending_qstt = []
                # KV-accumulation matmuls are likewise deferred one half:
                # they depend on the elementwise K/V chain (~4-5us deep) and
                # the PE instruction FIFO is strict in-order, so emitting them
                # immediately would head-of-line-block the next half's
                # independent projection matmuls.
                pending_kv = []

                def emit_kv(li0):
                    for jj2 in range(2):
                        li = li0 + jj2
                        for m in range(NCC):
                            ks = Kt_sb[:, li, 128 * m:128 * (m + 1)]
                            nc.tensor.matmul(
                                KV_ps[m], ks, Vt_sb[:, li, m, :],
                                start=(li == 0 and m % 2 == 0),
                                stop=(li == NL128 - 1))
                for j in range(NL512):
                    if j == 0:
                        xt = xt0
                    else:
                        xt = xin.tile([128, NCC, 512], bf16, name="xt", tag="xt")
                        nc.sync.dma_start(out=xt,
                                          in_=x_ap[:, :, 512 * j:512 * (j + 1)])

                    def kv_half(j, half, xt):
                        # Both 128-l chunks of this half processed as one
                        # [128, 2, 512] pair: halves the per-op fixed cost on
                        # every DVE/ACT instruction in the K/V chain.
                        li0 = 4 * j + 2 * half
                        kp = pkv.tile([128, 2, 512], f32, tag="kp", name="kp", bufs=1)
                        vp = pkv.tile([128, 2, 512], f32, tag="vp", name="vp", bufs=1)
                        for jj2 in range(2):
                            lf = 128 * (2 * half + jj2)
                            for cc in range(NCC):
                                xs = xt[:, cc, lf:lf + 128]
                                nc.tensor.matmul(kp[:, jj2, :], xs, wk_sb[:, cc, :],
                                                 start=(cc == 0), stop=(cc == NCC - 1))
                        for jj2 in range(2):
                            lf = 128 * (2 * half + jj2)
                            for cc in range(NCC):
                                xs = xt[:, cc, lf:lf + 128]
                                nc.tensor.matmul(vp[:, jj2, :], xs, wv_sb[:, cc, :],
                                                 start=(cc == 0), stop=(cc == NCC - 1))
                        # K^T = elu(kp + bk) + 1 = min(exp(t),1) + relu(t)
                        t0 = ev.tile([128, 2, 512], f32, tag="t0", bufs=3, name="t0")
                        e0 = ev.tile([128, 2, 512], bf16, tag="e0", bufs=4, name="e0")
                        r0 = ev.tile([128, 2, 512], bf16, tag="r0", bufs=4, name="r0")
                        nc.vector.tensor_add(t0, kp, bkb2)
                        # V^T = vp + bv  (written into the 129-strided layout)
                        nc.vector.tensor_add(
                            Vt_sb[:, li0:li0 + 2, :, 0:128],
                            vp.rearrange("p two (m c) -> p two m c", m=NCC),
                            bvb4x2)
                        nc.scalar.activation(e0, t0, AF.Exp)
                        nc.scalar.activation(r0, t0, AF.Relu)
                        nc.vector.scalar_tensor_tensor(
                            Kt_sb[:, li0:li0 + 2, :], e0, 1.0, r0, OP.min, OP.add)
                        pending_kv.append(li0)

                    def q_proj(j, op2, xt):
                        # Q projection, one oi-pair (spreads ACT load).  Two
                        # single-bank psum tags rotate so oi+1's matmuls can
                        # run while oi's ACT evictions still read their bank.
                        eq = ev.tile([128, 2, 512], bf16, tag="e0", bufs=4, name="eq")
                        rq = ev.tile([128, 2, 512], bf16, tag="r0", bufs=4, name="rq")
                        for oi2 in range(2):
                            oi = 2 * op2 + oi2
                            qp = pq.tile([128, 512], f32, tag=f"qp{oi2}",
                                         name=f"qp{oi2}")
                            for cc in range(NCC):
                                nc.tensor.matmul(
                                    qp,
                                    wq_sb[:, cc, 128 * oi:128 * (oi + 1)],
                                    xt[:, cc, :],
                                    start=(cc == 0), stop=(cc == NCC - 1))
                            nc.scalar.activation(eq[:, oi2, :], qp,
                                                 AF.Exp, bias=bq_sb[:, oi:oi + 1])
                            nc.scalar.activation(rq[:, oi2, :], qp,
                                                 AF.Relu, bias=bq_sb[:, oi:oi + 1])
                            if pending_qstt:
                                nc.vector.scalar_tensor_tensor(*pending_qstt.pop())
                            pending_qstt.append(
                                (Q_sb[:, oi, 512 * j:512 * (j + 1)],
                                 eq[:, oi2, :], 1.0, rq[:, oi2, :],
                                 OP.min, OP.add))

                    if j < NL512 - 1:
                        for half in range(2):
                            kv_half(j, half, xt)
                            if len(pending_kv) > 1:
                                emit_kv(pending_kv.pop(0))
                            q_proj(j, half, xt)
                    else:
                        # last chunk: Q first, so the dangling K/V eviction
                        # chain into the final KV matmuls is as short as
                        # possible before the phase-2 transition
                        q_proj(j, 0, xt)
                        q_proj(j, 1, xt)
                        kv_half(j, 0, xt)
                        emit_kv(pending_kv.pop(0))
                        kv_half(j, 1, xt)
                while pending_kv:
                    emit_kv(pending_kv.pop(0))

                # ---- evict KV (block-diag) and Ksum ----
                for m in range(NCC):
                    kv_m = KV_ps[m]
                    nc.vector.tensor_copy(KVbd_sb[0:64, m, 0:64], kv_m[0:64, 0:64])
                    nc.vector.tensor_copy(KVbd_sb[64:128, m, 64:128],
                                          kv_m[64:128, 64:128])
                    nc.vector.tensor_copy(ksum_sb[:, m:m + 1], kv_m[:, 128:129])
                for m in range(NCC):
                    nc.vector.tensor_scalar_mul(
                        KsumRep_sb[0:64, m, 0:64], ones128_sb[0:64, :],
                        ksum_sb[0:64, m:m + 1])
                    nc.vector.tensor_scalar_mul(
                        KsumRep_sb[64:128, m, 64:128], ones128_sb[64:128, :],
                        ksum_sb[64:128, m:m + 1])
                # the last Q combine is only consumed by phase-2 j=7 -- flush
                # it AFTER the KV eviction chain so it doesn't delay S_b(j=0)
                if pending_qstt:
                    nc.vector.scalar_tensor_tensor(*pending_qstt.pop())

            if debug_outputs:
                nc.sync.dma_start(out=dbg["Q"].ap(), in_=Q_sb)
                nc.sync.dma_start(out=dbg["Kt"].ap(), in_=Kt_sb)
                nc.sync.dma_start(out=dbg["Vt"].ap(), in_=Vt_sb)
                nc.sync.dma_start(out=dbg["KVbd"].ap(), in_=KVbd_sb)
                nc.sync.dma_start(out=dbg["KsumRep"].ap(), in_=KsumRep_sb)

            # ================= phase 2: attention out + O-projection ==========
            # The out matmuls read Q directly (no dependency on the reciprocal
            # chain); Z is applied by the DVE eviction multiply, so the PE
            # stream never waits on DVE except via psum slot reuse.
            with ExitStack() as p2stack:
                # PSUM: sb pair (2) + out pair (2) + y pair x2 bufs (4) = 8
                p2 = p2stack.enter_context(tc.tile_pool(name="p2", bufs=1, space="PSUM"))
                p2o = p2stack.enter_context(tc.tile_pool(name="p2o", bufs=2, space="PSUM"))
                zbp = p2stack.enter_context(tc.tile_pool(name="zbp", bufs=2))
                ytp = p2stack.enter_context(tc.tile_pool(name="ytp", bufs=2))

                def y_block(j, fine=False):
                    lsl = slice(512 * j, 512 * (j + 1))
                    for op2 in range(2):
                        yt = ytp.tile([128, 2, 512], f32, name="yt")
                        for oi2 in range(2):
                            oi = 2 * op2 + oi2
                            yp = p2o.tile([128, 512], f32, tag=f"y{oi2}",
                                          name="yp", bufs=2)
                            for vi in range(NCC):
                                nc.tensor.matmul(
                                    yp,
                                    wo_sb[:, vi, 128 * oi:128 * (oi + 1)],
                                    Om_sb[:, vi, lsl],
                                    start=(vi == 0), stop=(vi == NCC - 1))
                            nc.scalar.activation(yt[:, oi2, :], yp,
                                                 AF.Identity, bias=bo_sb[:, oi:oi + 1])
                            if fine:  # drain the tail with per-oi DMAs
                                nc.sync.dma_start(
                                    out=out_ap[128 * oi:128 * (oi + 1), lsl],
                                    in_=yt[:, oi2, :])
                        if not fine:
                            nc.sync.dma_start(
                                out=out_ap[256 * op2:256 * (op2 + 1), lsl].rearrange(
                                    "(two p) l -> p two l", p=128),
                                in_=yt)

                # y-block runs one j behind so its matmuls fill the PE while the
                # next j's recip/qz DVE chain is in flight.
                for j in range(NL512):
                    lsl = slice(512 * j, 512 * (j + 1))
                    for mp in range(2):
                        sbp = p2.tile([128, 2, 512], f32, tag="sb", name="sbp")
                        outp = p2.tile([128, 2, 512], f32, tag="out", name="outp")
                        for m2 in range(2):
                            m = 2 * mp + m2
                            nc.tensor.matmul(sbp[:, m2, :], KsumRep_sb[:, m, :],
                                             Q_sb[:, m, lsl], start=True, stop=True)
                            nc.tensor.matmul(outp[:, m2, :], KVbd_sb[:, m, :],
                                             Q_sb[:, m, lsl], start=True, stop=True)
                        zb = zbp.tile([128, 2, 512], f32)
                        # approx reciprocal (~18 bits, S~1e5 so no edge cases)
                        # is ~5x cheaper on DVE than the exact reciprocal()
                        nc.vector.reciprocal_approx_fast(zb, sbp)
                        nc.vector.tensor_mul(Om_sb[:, 2 * mp:2 * mp + 2, lsl],
                                             outp, zb)
                    if j > 0:
                        y_block(j - 1)
                y_block(NL512 - 1, fine=True)
                if debug_outputs:
                    nc.sync.dma_start(out=dbg["Om"].ap(), in_=Om_sb)

    nc.compile()
    return nc


def _get_nc():
    if "nc" not in _CACHE:
        _CACHE["nc"] = _build_nc()
    return _CACHE["nc"]


def _make_in_maps(inputs):
    x = np.asarray(inputs["x"], dtype=np.float32)
    wq = np.asarray(inputs["wq"], dtype=np.float32)
    wk = np.asarray(inputs["wk"], dtype=np.float32)
    wv = np.asarray(inputs["wv"], dtype=np.float32)
    wo = np.asarray(inputs["wo"], dtype=np.float32)
    bq = np.asarray(inputs["bq"], dtype=np.float32)
    bk = np.asarray(inputs["bk"], dtype=np.float32)
    bv = np.asarray(inputs["bv"], dtype=np.float32)
    bo = np.asarray(inputs["bo"], dtype=np.float32)

    shared = {
        "wqT": np.ascontiguousarray(wq.T).astype(BF16),
        "wkT": np.ascontiguousarray(wk.T).astype(BF16),
        "wvT": np.ascontiguousarray(wv.T).astype(BF16),
        "woT": np.ascontiguousarray(wo.T).astype(BF16),
        "bqT": np.ascontiguousarray(bq.reshape(NCC, 128).T),
        "boT": np.ascontiguousarray(bo.reshape(NCC, 128).T),
        "bkb": np.ascontiguousarray(np.broadcast_to(bk, (128, C))),
        "bvb": np.ascontiguousarray(np.broadcast_to(bv, (128, C))),
    }
    in_maps = []
    for b in range(NB):
        m = dict(shared)
        m["x"] = np.ascontiguousarray(x[b].reshape(C, L)).astype(BF16)
        in_maps.append(m)
    return in_maps


def _run(inputs, trace=False):
    from concourse.bass_utils import run_bass_kernel_spmd

    nc = _get_nc()
    in_maps = _make_in_maps(inputs)
    res = run_bass_kernel_spmd(nc, in_maps, core_ids=list(range(NB)), trace=trace)
    outs = np.stack([np.asarray(res.results[b]["out"], dtype=np.float32)
                     for b in range(NB)])
    y = outs.reshape(NB, C, 64, 64)
    return y, res


def kernel(**inputs) -> np.ndarray:
    y, _ = _run(inputs, trace=False)
    return y

